# revision 30
# baseline (speedup 1.0000x reference)
"""Trainium2 Bass kernel for nn_BoxEncoder (B=128, T=200, NC=3, NB=2, D=512, DH=256).

Strategy (data-parallel over batch, 16 batch items per core x 8 cores):

 - The within-camera argsort over NB=2 boxes reduces to a single
   compare-and-swap (stable sort of 2 keys).
 - All per-box scalars are computed as [128, n] "feature planes" with DVE
   ops, laid out 32 feature-columns per token in a big bf16 T_feat tile
   (box slots j=0..149 first, dist slots j=150..224).
 - PE transposes of [128,128] chunks of T_feat produce feature-major lhsT
   tiles (32-aligned partition bases) feeding the matmuls.
 - LayerNorm stats via a Gram-matrix trick: var = x.(x@G) - mu^2 with
   G = W1@W1^T/256, computed by 38 block-diagonal matmuls (N=132) plus
   DVE segmented multiply-reduce -- no second z pass, no bn_stats.
 - Both Sqrt batches (dist feature + LN rstd) run before any GELU so the
   ACT spline table switches exactly once.
 - P5 is software-pipelined with stage offsets (z:+3, gelu:+2,
   transpose:+1, out:+0) so every cross-engine dependency is satisfied a
   full iteration ahead -- the PE issues back-to-back matmuls, keeping
   the HAM clock-gate warm (2.4 GHz).
 - Outputs are staged and DMA'd as bf16 (halved HBM traffic; host
   upcasts), box rows on the sync queue, dist rows on the gpsimd queue.
 - Missing boxes produce exactly missing_emb through the matmul (their
   geom path contributes gelu(0)=0), so no select/where is needed.

Token layout per core: partition p = bt*8 + q (bt = batch item 0..15,
q = 0..7). Box slot j in [0,150) covers output rows bt*1800 + 600 +
q*150 + j; dist slot j in [150,225) covers rows bt*1800 + q*75 + (j-150).
"""

import numpy as np
import ml_dtypes

B, T, NCAM, NB, D, DH = 128, 200, 3, 2, 512, 256
IW, IH = 640.0, 400.0
NCORES = 8
BPC = B // NCORES            # batch items per core
JB, JD = 150, 75             # box / dist j-slots per partition
J = JB + JD                  # 225
F = 32                       # feature columns per j-slot
NCHUNK = (J * F + 127) // 128   # 57 transpose chunks (56 full + 1 of 32 cols)
NG = (JB + 3) // 4           # 38 stats chunks (chunk 37 partly dist, harmless)

_CACHE = {}

# bf16 pack column offsets
C_ID = 0
C_W1 = C_ID + 128
C_W2HI = C_W1 + 256
C_W2LO = C_W2HI + 512
C_W2X = C_W2LO + 512          # 3 cam variants, 512 each
C_GBLK = C_W2X + 3 * 512
C_WREP = C_GBLK + 132         # dist_w replicated over partitions
C_BREP = C_WREP + 512         # dist_b replicated over partitions
NBF = C_BREP + 512


def _build_nc(dist_b_zero=True):
    from contextlib import ExitStack
    import concourse.bacc as bacc
    import concourse.mybir as mybir
    import concourse.tile as tile

    f32 = mybir.dt.float32
    bf16 = mybir.dt.bfloat16
    A = mybir.AluOpType
    AF = mybir.ActivationFunctionType
    AX = mybir.AxisListType

    nc = bacc.Bacc("TRN2", target_bir_lowering=False, debug=False,
                   num_devices=NCORES)
    fpk = nc.declare_dram_parameter("fpk", [128, 900], f32, isOutput=False)
    bpk = nc.declare_dram_parameter("bpk", [128, NBF], bf16, isOutput=False)
    out_d = nc.declare_dram_parameter("out", [BPC, 1800, D], bf16, isOutput=True)

    with ExitStack() as ctx:
        tc = ctx.enter_context(tile.TileContext(nc))
        cp = ctx.enter_context(tc.tile_pool(name="const", bufs=1))
        sc = ctx.enter_context(tc.tile_pool(name="scratch", bufs=1))
        pp = ctx.enter_context(tc.tile_pool(name="ps", bufs=1, space="PSUM"))
        hp = ctx.enter_context(tc.tile_pool(name="hsb", bufs=4))
        tsp = ctx.enter_context(tc.tile_pool(name="htsb", bufs=4))
        bstg = ctx.enter_context(tc.tile_pool(name="bstage", bufs=2))
        dstg = ctx.enter_context(tc.tile_pool(name="dstage", bufs=2))

        fpack = cp.tile([128, 900], f32)
        nc.sync.dma_start(fpack[:], fpk[:])
        bpack = cp.tile([128, NBF], bf16)
        nc.sync.dma_start(bpack[:], bpk[:])

        raw = fpack[:, 0:900]
        idb = bpack[:, C_ID:C_ID + 128]
        w1 = bpack[:, C_W1:C_W1 + 256]
        w2hi = bpack[:, C_W2HI:C_W2HI + 512]
        w2lo = bpack[:, C_W2LO:C_W2LO + 512]
        w2x = [bpack[:, C_W2X + c * 512: C_W2X + (c + 1) * 512] for c in range(3)]
        gblk = bpack[:, C_GBLK:C_GBLK + 132]
        wrep = bpack[:, C_WREP:C_WREP + 512]
        brep = bpack[:, C_BREP:C_BREP + 512]

        TF = cp.tile([128, J * F], bf16)
        nc.gpsimd.memset(TF[:], 0.0)

        TFj = TF.rearrange("p (j f) -> p j f", f=F)
        TFb = TFj[:, :JB, :]                       # box slots (first now)
        TFd = TFj[:, JB:, :]                       # dist slots
        TFbp = TF[:, : JB * F].rearrange("p (m g f) -> p m g f", g=2, f=F)
        raw6 = raw.rearrange("p (b s) -> p b s", s=6)
        rawp = raw.rearrange("p (m g s) -> p m g s", g=2, s=6)

        # ---------------- P1: feature planes ----------------
        sPres = sc.tile([128, JB], f32)
        sKey = sc.tile([128, JB], f32)
        sSwap = sc.tile([128, JD], f32)
        sD = sc.tile([128, JD], f32)
        sSD = sc.tile([128, JD], f32)
        sw = [sc.tile([128, JB], f32, tag=f"swp{i}", name=f"swp{i}")
              for i in range(6)]
        sT0 = sc.tile([128, JB], f32)
        sT1 = sc.tile([128, JB], f32)

        nc.vector.tensor_tensor(sT0[:], raw6[:, :, 0], raw6[:, :, 1], A.add)
        nc.vector.tensor_tensor(sT1[:], raw6[:, :, 2], raw6[:, :, 3], A.add)
        nc.vector.tensor_tensor(sT0[:], sT0[:], sT1[:], A.add)
        nc.vector.tensor_scalar(sPres[:], sT0[:], 0.0, None, A.not_equal)
        # key = cat - 1000*pres  (order-equivalent to cat + 1000*(1-pres))
        nc.vector.scalar_tensor_tensor(sKey[:], sPres[:], -1000.0,
                                       raw6[:, :, 4], A.mult, A.add)
        sKeyp = sKey.rearrange("p (m g) -> p m g", g=2)
        nc.vector.tensor_tensor(sSwap[:], sKeyp[:, :, 1], sKeyp[:, :, 0], A.is_lt)

        # compare-and-swap each of the 6 raw components + presence
        for i in range(6):
            ve, vo = rawp[:, :, 0, i], rawp[:, :, 1, i]
            dst = sw[i].rearrange("p (m g) -> p m g", g=2)
            nc.vector.tensor_tensor(sD[:], vo, ve, A.subtract)
            nc.vector.tensor_tensor(sSD[:], sD[:], sSwap[:], A.mult)
            nc.vector.tensor_tensor(dst[:, :, 0], ve, sSD[:], A.add)
            nc.vector.tensor_tensor(dst[:, :, 1], vo, sSD[:], A.subtract)
        sPresP = sPres.rearrange("p (m g) -> p m g", g=2)
        nc.vector.tensor_tensor(sD[:], sPresP[:, :, 1], sPresP[:, :, 0], A.subtract)
        nc.vector.tensor_tensor(sSD[:], sD[:], sSwap[:], A.mult)
        nc.vector.tensor_tensor(TFbp[:, :, 0, 14], sPresP[:, :, 0], sSD[:], A.add)
        nc.vector.tensor_tensor(TFbp[:, :, 1, 14], sPresP[:, :, 1], sSD[:], A.subtract)

        sX1, sY1, sX2, sY2, sCat, sConf = sw
        # normalized coords in f32 scratch (reused for derived features)
        sx1n = sc.tile([128, JB], f32)
        sy1n = sc.tile([128, JB], f32)
        sx2n = sc.tile([128, JB], f32)
        sy2n = sc.tile([128, JB], f32)
        sWf = sc.tile([128, JB], f32)
        sHf = sc.tile([128, JB], f32)
        sCX = sc.tile([128, JB], f32)
        sCY = sc.tile([128, JB], f32)
        nc.vector.tensor_scalar(sx1n[:], sX1[:], 1.0 / IW, None, A.mult)
        nc.vector.tensor_scalar(sy1n[:], sY1[:], 1.0 / IH, None, A.mult)
        nc.vector.tensor_scalar(sx2n[:], sX2[:], 1.0 / IW, None, A.mult)
        nc.vector.tensor_scalar(sy2n[:], sY2[:], 1.0 / IH, None, A.mult)
        # f0..f3 normalized coords (bf16 casts)
        nc.vector.tensor_copy(TFb[:, :, 0], sx1n[:])
        nc.vector.tensor_copy(TFb[:, :, 1], sy1n[:])
        nc.vector.tensor_copy(TFb[:, :, 2], sx2n[:])
        nc.vector.tensor_copy(TFb[:, :, 3], sy2n[:])
        # f4 w, f5 h, f6 cx*2, f7 cy*2 (the 0.5 is folded into the weights)
        nc.vector.tensor_tensor(sWf[:], sx2n[:], sx1n[:], A.subtract)
        nc.vector.tensor_tensor(sHf[:], sy2n[:], sy1n[:], A.subtract)
        nc.vector.tensor_tensor(sCX[:], sx1n[:], sx2n[:], A.add)
        nc.vector.tensor_tensor(sCY[:], sy1n[:], sy2n[:], A.add)
        nc.vector.tensor_copy(TFb[:, :, 4], sWf[:])
        nc.vector.tensor_copy(TFb[:, :, 5], sHf[:])
        nc.vector.tensor_copy(TFb[:, :, 6], sCX[:])
        nc.vector.tensor_copy(TFb[:, :, 7], sCY[:])
        # f8 area, f9 aspect = w / (h + 1e-6)
        nc.vector.tensor_tensor(TFb[:, :, 8], sWf[:], sHf[:], A.mult)
        nc.vector.tensor_scalar(sT0[:], sHf[:], 1e-6, None, A.add)
        nc.vector.reciprocal(sT1[:], sT0[:])
        nc.vector.tensor_tensor(TFb[:, :, 9], sWf[:], sT1[:], A.mult)
        # f10..12 cat one-hots * pres ; f13 conf*pres ; f15 = 1-pres
        for k in range(3):
            nc.vector.scalar_tensor_tensor(TFb[:, :, 10 + k], sCat[:], float(k),
                                           TFb[:, :, 14], A.is_equal, A.mult)
        nc.vector.tensor_tensor(TFb[:, :, 13], sConf[:], TFb[:, :, 14], A.mult)
        nc.vector.tensor_scalar(TFb[:, :, 15], TFb[:, :, 14], -1.0, 1.0,
                                A.mult, A.add)
        # dist tokens: f16 = 0.5*sqrt(dx2^2+dy2^2) (cx stored doubled), f17 = 1
        sDx = sc.tile([128, JD], f32)
        sDy = sc.tile([128, JD], f32)
        sCXp = sCX.rearrange("p (m g) -> p m g", g=2)
        sCYp = sCY.rearrange("p (m g) -> p m g", g=2)
        nc.vector.tensor_tensor(sDx[:], sCXp[:, :, 0], sCXp[:, :, 1], A.subtract)
        nc.vector.tensor_tensor(sDy[:], sCYp[:, :, 0], sCYp[:, :, 1], A.subtract)
        nc.vector.tensor_tensor(sDx[:], sDx[:], sDx[:], A.mult)
        nc.vector.tensor_tensor(sDy[:], sDy[:], sDy[:], A.mult)
        nc.vector.tensor_tensor(sDx[:], sDx[:], sDy[:], A.add)
        sDist = sc.tile([128, JD], f32)
        nc.scalar.activation(sDist[:], sDx[:], AF.Sqrt, scale=0.25)
        nc.vector.tensor_copy(TFd[:, :, 16], sDist[:])
        nc.vector.memset(TFd[:, :, 17], 1.0)

        # persistent PSUM block tiles: multiple logical slots packed per bank
        # (subtile dep tracking gives per-slot semaphores) so the PE can run
        # several iterations ahead of its consumers
        opairs = [pp.tile([128, 2 * D], f32, tag=f"op{i}", name=f"op{i}")
                  for i in range(2)]                                     # 4 banks
        zs = [pp.tile([128, DH], f32, tag=f"z{i}", name=f"z{i}")
              for i in range(2)]                                         # 2 banks
        hts_ps = [pp.tile([128, DH], bf16, tag=f"ht{i}", name=f"ht{i}")
                  for i in range(2)]                                     # 2 banks

        def zsl(k):
            return zs[k % 2][:]

        def htsl(k):
            return hts_ps[k % 2][:]

        def osl(k):
            return opairs[(k // 2) % 2][:, (k % 2) * D:(k % 2 + 1) * D]

        # ---------------- P2: transpose T_feat chunks -> bf16 lhsT tiles ----
        cta = cp.tile([128, NCHUNK * 128], bf16)
        for ci in range(NCHUNK):
            w_cols = min(128, J * F - ci * 128)
            ps = htsl(ci)[:, 0:128]
            nc.tensor.transpose(ps[:w_cols, :], TF[:, ci * 128: ci * 128 + w_cols],
                                idb)
            dst = cta[:w_cols, ci * 128: ci * 128 + 128]
            if ci % 2 == 0:
                nc.vector.tensor_copy(dst, ps[:w_cols, :])
            else:
                nc.scalar.copy(dst, ps[:w_cols, :])

        def lhsT(j):
            ci, jj = j // 4, j % 4
            return cta[32 * jj: 32 * jj + 32, ci * 128: (ci + 1) * 128]

        # ---------------- P3: LN stats via Gram trick ----------------
        VAR = cp.tile([128, NG * 4], f32)
        MU = cp.tile([128, NG * 4], f32)
        tmps = [sc.tile([128, 128], f32, tag=f"tmp{i}", name=f"tmp{i}")
                for i in range(2)]
        for g in range(NG):
            y = zsl(g)[:, 0:132]
            nc.tensor.matmul(y[:], cta[:, g * 128:(g + 1) * 128], gblk,
                             start=True, stop=True)
            # full-width contiguous ops (pad feature cols are zero, so they
            # contribute nothing to the reduction)
            tmp = tmps[g % 2]
            nc.vector.tensor_tensor(tmp[:], TF[:, g * 128:(g + 1) * 128],
                                    y[:, 0:128], A.mult)
            nc.vector.tensor_reduce(VAR[:, 4 * g: 4 * g + 4],
                                    tmp.rearrange("p (s f) -> p s f", f=32),
                                    AX.X, A.add)
            nc.scalar.copy(MU[:, 4 * g: 4 * g + 4], y[:, 128:132])

        # ---------------- P3b: batched rstd (single Sqrt region) ------------
        sdv = cp.tile([128, NG * 4], f32)
        eps = cp.tile([128, 1], f32)
        nc.vector.memset(eps[:], 1e-5)
        nc.vector.tensor_tensor(sdv[:], MU[:], MU[:], A.mult)
        nc.vector.tensor_tensor(VAR[:], VAR[:], sdv[:], A.subtract)
        nc.scalar.activation(sdv[:], VAR[:], AF.Sqrt, bias=eps[:])
        rstd = cp.tile([128, NG * 4], f32)
        nc.vector.reciprocal(rstd[:], sdv[:])
        nmr = cp.tile([128, NG * 4], f32)
        nc.vector.scalar_tensor_tensor(nmr[:], MU[:], -1.0, rstd[:],
                                       A.mult, A.mult)

        # ---------------- P5: pipelined box tiles + interleaved dist --------
        vd = out_d[:, 0:600, :].rearrange("b (q r) d -> b q r d", q=8)
        vb = out_d[:, 600:1800, :].rearrange("b (q r) d -> b q r d", q=8)
        h_tiles, hts_tiles = {}, {}
        state = {"box_stage": None, "stage_fill": 0, "box_dma": 0,
                 "dist_stage": None, "dist_fill": 0}

        def emit_cast(p):
            # paired PSUM->bf16 cast for box tiles (2p, 2p+1), one iteration
            # after the pair's accumulation completed (no head-of-line wait)
            if state["box_stage"] is None:
                state["box_stage"] = bstg.tile([128, 8 * D], bf16, tag="bstage",
                                               name="box_stage")
                state["stage_fill"] = 0
            sf = state["stage_fill"]
            dstc = state["box_stage"][:, sf * D:(sf + 2) * D]
            src = opairs[p % 2][:]
            if p % 2 == 0:
                nc.vector.tensor_copy(dstc, src)
            else:
                nc.scalar.copy(dstc, src)
            state["stage_fill"] += 2
            if state["stage_fill"] == 8 or p == JB // 2 - 1:
                j0 = 2 * p - state["stage_fill"] + 2
                eng = nc.sync if state["box_dma"] % 2 == 0 else nc.gpsimd
                eng.dma_start(vb[:, :, j0:j0 + state["stage_fill"], :],
                              state["box_stage"][:, : state["stage_fill"] * D])
                state["box_dma"] += 1
                state["box_stage"] = None

        def emit_dist(d):
            # dist tokens are rank-1 (dist*w + b): one 4x-tier DVE
            # tensor_scalar straight into SBUF staging -- no PE, no PSUM
            if state["dist_stage"] is None:
                state["dist_stage"] = dstg.tile([128, 4 * D], bf16, tag="dstage",
                                                name="dist_stage")
                state["dist_fill"] = 0
            df = state["dist_fill"]
            ddst = state["dist_stage"][:, df * D:(df + 1) * D]
            nc.vector.tensor_scalar(ddst, wrep, sDist[:, d:d + 1],
                                    None, A.mult)
            if not dist_b_zero:
                nc.vector.tensor_tensor(ddst, ddst, brep, A.add)
            state["dist_fill"] += 1
            if state["dist_fill"] == 4 or d == JD - 1:
                d0 = d - state["dist_fill"] + 1
                nc.sync.dma_start(vd[:, :, d0:d0 + state["dist_fill"], :],
                                  state["dist_stage"][:, : state["dist_fill"] * D])
                state["dist_stage"] = None

        for k in range(-3, JB + 2):
            kz, kh, kt, kc, ko = k + 3, k + 2, k + 1, k + 1, k
            if 0 <= kz < JB:
                jj = kz % 4
                nc.tensor.matmul(zsl(kz), lhsT(kz), w1[32 * jj: 32 * jj + 32, :],
                                 start=True, stop=True,
                                 tile_position=(32 * jj, 0))
            if 0 <= kh < JB:
                h = hp.tile([128, DH], bf16, tag="h", name="h")
                nc.scalar.activation(h[:], zsl(kh), AF.Gelu,
                                     bias=nmr[:, kh:kh + 1],
                                     scale=rstd[:, kh:kh + 1])
                h_tiles[kh] = h
            if 0 <= kt < JB:
                ht = htsl(kt)
                hin = h_tiles.pop(kt)
                nc.tensor.transpose(ht[:, 0:128], hin[:, 0:128], idb)
                nc.tensor.transpose(ht[:, 128:256], hin[:, 128:256], idb)
            if 0 <= kc < JB:
                hts = tsp.tile([128, DH], bf16, tag="hts", name="hts")
                nc.vector.tensor_copy(hts[:], htsl(kc))
                hts_tiles[kc] = hts
            if 0 <= ko < JB:
                jj = ko % 4
                cam = (ko % 6) // 2
                hs = hts_tiles.pop(ko)
                o = osl(ko)
                nc.tensor.matmul(o, hs[:, 0:128], w2hi, start=True, stop=False)
                nc.tensor.matmul(o, hs[:, 128:256], w2lo, start=False, stop=False)
                nc.tensor.matmul(o, lhsT(ko), w2x[cam][32 * jj: 32 * jj + 32, :],
                                 start=False, stop=True,
                                 tile_position=(32 * jj, 0))
                if ko % 2 == 0 and ko // 2 < JD:
                    emit_dist(ko // 2)
            if 2 <= ko <= JB and ko % 2 == 0:
                emit_cast((ko - 2) // 2)

    nc.compile()
    return nc


def _prep_inputs(inputs):
    f32 = np.float32
    bf = ml_dtypes.bfloat16
    scale = float(np.asarray(inputs["scale"]))

    W1p = np.zeros((32, DH), f32)
    W1p[0:10] = np.asarray(inputs["geom_w1"], f32)
    W1p[6] *= 0.5
    W1p[7] *= 0.5
    w1rep = np.tile(W1p, (4, 1))

    W2s = scale * np.asarray(inputs["geom_w2"], f32)
    w2hi, w2lo = W2s[:128], W2s[128:]

    cat_t = np.asarray(inputs["cat_table"], f32)
    cam_t = np.asarray(inputs["cam_table"], f32)
    bias_row = (np.asarray(inputs["geom_b2"], f32)
                + np.asarray(inputs["conf_b"], f32)
                + np.asarray(inputs["center_b"], f32))
    w2x_reps = []
    for c in range(3):
        W2x = np.zeros((32, D), f32)
        W2x[6] = scale * np.asarray(inputs["center_w"], f32)[0] * 0.5
        W2x[7] = scale * np.asarray(inputs["center_w"], f32)[1] * 0.5
        W2x[10:13] = scale * cat_t
        W2x[13] = scale * np.asarray(inputs["conf_w"], f32)[0]
        W2x[14] = scale * (bias_row + cam_t[c])
        W2x[15] = np.asarray(inputs["missing_emb"], f32)[0]
        W2x[16] = np.asarray(inputs["dist_w"], f32)[0]
        W2x[17] = np.asarray(inputs["dist_b"], f32)
        w2x_reps.append(np.tile(W2x, (4, 1)))

    G = (W1p @ W1p.T) / 256.0
    w_mu = W1p.sum(axis=1) / 256.0
    gblk = np.zeros((128, 132), f32)
    for s in range(4):
        gblk[32 * s:32 * s + 32, 32 * s:32 * s + 32] = G
        gblk[32 * s:32 * s + 32, 128 + s] = w_mu

    idb = np.eye(128, dtype=f32)
    wrep = np.tile(np.asarray(inputs["dist_w"], f32), (128, 1))
    brep = np.tile(np.asarray(inputs["dist_b"], f32)[None, :], (128, 1))
    bpk = np.concatenate(
        [idb, w1rep, w2hi, w2lo] + w2x_reps + [gblk, wrep, brep], axis=1
    ).astype(bf)

    box = np.asarray(inputs["box_data"], f32)
    fpks = []
    for c in range(NCORES):
        rawc = box[c * BPC:(c + 1) * BPC].reshape(BPC, T * 6, 6)
        rawc = rawc.reshape(BPC, 8, JB, 6).reshape(128, 900)
        fpks.append(np.ascontiguousarray(rawc, dtype=f32))
    return fpks, bpk


def _fast_path_ok(inputs):
    try:
        shapes = {
            "box_data": (B, T, 6, 6), "cat_table": (3, D), "geom_w1": (10, DH),
            "geom_b1": (DH,), "ln_g": (DH,), "ln_b": (DH,), "geom_w2": (DH, D),
            "geom_b2": (D,), "conf_w": (1, D), "conf_b": (D,),
            "center_w": (2, D), "center_b": (D,), "missing_emb": (1, D),
            "dist_w": (1, D), "dist_b": (D,), "cam_table": (NCAM, D),
        }
        for k, s in shapes.items():
            if tuple(np.asarray(inputs[k]).shape) != s:
                return False
        if not np.all(np.asarray(inputs["geom_b1"]) == 0):
            return False
        if not np.all(np.asarray(inputs["ln_g"]) == 1):
            return False
        if not np.all(np.asarray(inputs["ln_b"]) == 0):
            return False
        return True
    except Exception:
        return False


def _numpy_fallback(inputs):
    # Exact (slow) port of the reference for unexpected inputs.
    import math
    f32 = np.float32
    inp = {k: np.asarray(v) for k, v in inputs.items()}
    coords = inp["box_data"][..., :4].astype(f32)
    category = inp["box_data"][..., 4].astype(np.int32)
    conf = inp["box_data"][..., 5].astype(f32)
    norm = np.array([IW, IH, IW, IH], f32)
    cn = (coords / norm).reshape(B, T, NCAM, NB, 4)
    category = category.reshape(B, T, NCAM, NB)
    conf = conf.reshape(B, T, NCAM, NB, 1)
    presence = (cn.sum(-1) != 0).astype(f32)
    sort_key = category.astype(f32) + (1.0 - presence) * 1000.0
    idx = np.argsort(sort_key, axis=-1, kind="stable")
    cn = np.take_along_axis(cn, idx[..., None], axis=-2)
    category = np.take_along_axis(category, idx, axis=-1)
    conf = np.take_along_axis(conf, idx[..., None], axis=-2)
    presence = (cn.sum(-1) != 0).astype(f32)[..., None]
    x1, y1, x2, y2 = cn[..., 0], cn[..., 1], cn[..., 2], cn[..., 3]
    w, h = x2 - x1, y2 - y1
    cx, cy = (x1 + x2) * 0.5, (y1 + y2) * 0.5
    area, aspect = w * h, w / (h + 1e-6)
    dx, dy = cx[..., 0] - cx[..., 1], cy[..., 0] - cy[..., 1]
    dist = np.sqrt(dx * dx + dy * dy)[..., None]
    dist_tok = dist @ inp["dist_w"].astype(f32) + inp["dist_b"].astype(f32)
    geom = np.stack([x1, y1, x2, y2, w, h, cx, cy, area, aspect], axis=-1)
    z = geom @ inp["geom_w1"].astype(f32) + inp["geom_b1"].astype(f32)
    mu = z.mean(-1, keepdims=True)
    var = ((z - mu) ** 2).mean(-1, keepdims=True)
    xh = (z - mu) / np.sqrt(var + 1e-5) * inp["ln_g"].astype(f32) + inp["ln_b"].astype(f32)
    try:
        from scipy.special import erf as _erf
        g = xh * 0.5 * (1.0 + _erf(xh / np.sqrt(2.0)))
    except Exception:
        verf = np.vectorize(math.erf)
        g = xh * 0.5 * (1.0 + verf(xh / np.sqrt(2.0)))
    geom_p = g @ inp["geom_w2"].astype(f32) + inp["geom_b2"].astype(f32)
    cat_emb = inp["cat_table"].astype(f32)[category]
    conf_p = conf @ inp["conf_w"].astype(f32) + inp["conf_b"].astype(f32)
    center_p = np.stack([cx, cy], axis=-1) @ inp["center_w"].astype(f32) + inp["center_b"].astype(f32)
    cam_emb = inp["cam_table"].astype(f32).reshape(1, 1, NCAM, 1, D)
    tok = (geom_p + cat_emb + conf_p + center_p + cam_emb) * float(inp["scale"])
    tok = np.where(presence == 0, inp["missing_emb"].astype(f32)[0], tok)
    out = np.concatenate([dist_tok.reshape(B, T * NCAM, D),
                          tok.reshape(B, T * NCAM * NB, D)], axis=1)
    return out.astype(np.float32)


def _run(inputs, trace=False, tmpdir=None):
    from concourse.bass_utils import run_bass_kernel_spmd

    dbz = bool(np.all(np.asarray(inputs["dist_b"]) == 0))
    key = ("nc", dbz)
    if key not in _CACHE:
        _CACHE[key] = _build_nc(dist_b_zero=dbz)
    nc = _CACHE[key]

    fpks, bpk = _prep_inputs(inputs)
    in_maps = [{"fpk": fpks[c], "bpk": bpk} for c in range(NCORES)]
    res = run_bass_kernel_spmd(nc, in_maps, list(range(NCORES)),
                               trace=trace, tmpdir=tmpdir)
    out = np.concatenate([np.asarray(res.results[c]["out"])
                          for c in range(NCORES)], axis=0)
    return out.astype(np.float32), res


def kernel(**inputs):
    if not _fast_path_ok(inputs):
        return _numpy_fallback(inputs)
    out, _ = _run(inputs)
    return out


if __name__ == "__main__":
    import reference as ref
    inputs = {k: np.asarray(v) for k, v in ref.setup_inputs().items()}
    got = kernel(**inputs)
    exp = np.load("/tmp/expected.npy")
    d = got - exp
    print("rel fro:", np.linalg.norm(d) / np.linalg.norm(exp))
    print("absmax rel:", np.abs(d).max() / np.abs(exp).max())


# revision 33
# speedup vs baseline: 1.0260x; 1.0260x over previous
"""Trainium2 Bass kernel for nn_BoxEncoder (B=128, T=200, NC=3, NB=2, D=512, DH=256).

Strategy (data-parallel over batch, 16 batch items per core x 8 cores):

 - The within-camera argsort over NB=2 boxes reduces to a single
   compare-and-swap (stable sort of 2 keys).
 - All per-box scalars are computed as [128, n] "feature planes" with DVE
   ops, laid out 32 feature-columns per token in a big bf16 T_feat tile
   (box slots j=0..149 first, dist slots j=150..224).
 - PE transposes of [128,128] chunks of T_feat produce feature-major lhsT
   tiles (32-aligned partition bases) feeding the matmuls.
 - LayerNorm stats via a Gram-matrix trick: var = x.(x@G) - mu^2 with
   G = W1@W1^T/256, computed by 38 block-diagonal matmuls (N=132) plus
   DVE segmented multiply-reduce -- no second z pass, no bn_stats.
 - Both Sqrt batches (dist feature + LN rstd) run before any GELU so the
   ACT spline table switches exactly once.
 - P5 is software-pipelined with stage offsets (z:+3, gelu:+2,
   transpose:+1, out:+0) so every cross-engine dependency is satisfied a
   full iteration ahead -- the PE issues back-to-back matmuls, keeping
   the HAM clock-gate warm (2.4 GHz).
 - Outputs are staged and DMA'd as bf16 (halved HBM traffic; host
   upcasts), box rows on the sync queue, dist rows on the gpsimd queue.
 - Missing boxes produce exactly missing_emb through the matmul (their
   geom path contributes gelu(0)=0), so no select/where is needed.

Token layout per core: partition p = bt*8 + q (bt = batch item 0..15,
q = 0..7). Box slot j in [0,150) covers output rows bt*1800 + 600 +
q*150 + j; dist slot j in [150,225) covers rows bt*1800 + q*75 + (j-150).
"""

import numpy as np
import ml_dtypes

B, T, NCAM, NB, D, DH = 128, 200, 3, 2, 512, 256
IW, IH = 640.0, 400.0
NCORES = 8
BPC = B // NCORES            # batch items per core
JB, JD = 150, 75             # box / dist j-slots per partition
J = JB + JD                  # 225
F = 32                       # feature columns per j-slot
NCHUNK = (J * F + 127) // 128   # 57 transpose chunks (56 full + 1 of 32 cols)
NG = (JB + 3) // 4           # 38 stats chunks (chunk 37 partly dist, harmless)

_CACHE = {}

# bf16 pack column offsets
C_ID = 0
C_W1 = C_ID + 128
C_W2HI = C_W1 + 256
C_W2LO = C_W2HI + 512
C_W2X = C_W2LO + 512          # 3 cam variants, 512 each
C_GBLK = C_W2X + 3 * 512
C_WREP = C_GBLK + 132         # dist_w replicated over partitions
C_BREP = C_WREP + 512         # dist_b replicated over partitions
NBF = C_BREP + 512


def _build_nc(dist_b_zero=True):
    from contextlib import ExitStack
    import concourse.bacc as bacc
    import concourse.mybir as mybir
    import concourse.tile as tile

    f32 = mybir.dt.float32
    bf16 = mybir.dt.bfloat16
    A = mybir.AluOpType
    AF = mybir.ActivationFunctionType
    AX = mybir.AxisListType

    nc = bacc.Bacc("TRN2", target_bir_lowering=False, debug=False,
                   num_devices=NCORES)
    fpk = nc.declare_dram_parameter("fpk", [128, 900], f32, isOutput=False)
    bpk = nc.declare_dram_parameter("bpk", [128, NBF], bf16, isOutput=False)
    out_d = nc.declare_dram_parameter("out", [BPC, 1800, D], bf16, isOutput=True)

    with ExitStack() as ctx:
        tc = ctx.enter_context(tile.TileContext(nc))
        cp = ctx.enter_context(tc.tile_pool(name="const", bufs=1))
        sc = ctx.enter_context(tc.tile_pool(name="scratch", bufs=1))
        pp = ctx.enter_context(tc.tile_pool(name="ps", bufs=1, space="PSUM"))
        hp = ctx.enter_context(tc.tile_pool(name="hsb", bufs=4))
        tsp = ctx.enter_context(tc.tile_pool(name="htsb", bufs=4))
        bstg = ctx.enter_context(tc.tile_pool(name="bstage", bufs=2))
        dstg = ctx.enter_context(tc.tile_pool(name="dstage", bufs=2))

        fpack = cp.tile([128, 900], f32)
        nc.sync.dma_start(fpack[:], fpk[:])
        bpack = cp.tile([128, NBF], bf16)
        nc.sync.dma_start(bpack[:], bpk[:])

        raw = fpack[:, 0:900]
        idb = bpack[:, C_ID:C_ID + 128]
        w1 = bpack[:, C_W1:C_W1 + 256]
        w2hi = bpack[:, C_W2HI:C_W2HI + 512]
        w2lo = bpack[:, C_W2LO:C_W2LO + 512]
        w2x = [bpack[:, C_W2X + c * 512: C_W2X + (c + 1) * 512] for c in range(3)]
        gblk = bpack[:, C_GBLK:C_GBLK + 132]
        wrep = bpack[:, C_WREP:C_WREP + 512]
        brep = bpack[:, C_BREP:C_BREP + 512]

        TF = cp.tile([128, J * F], bf16)
        nc.gpsimd.memset(TF[:], 0.0)

        TFj = TF.rearrange("p (j f) -> p j f", f=F)
        TFb = TFj[:, :JB, :]                       # box slots (first now)
        TFd = TFj[:, JB:, :]                       # dist slots
        TFbp = TF[:, : JB * F].rearrange("p (m g f) -> p m g f", g=2, f=F)
        raw6 = raw.rearrange("p (b s) -> p b s", s=6)
        rawp = raw.rearrange("p (m g s) -> p m g s", g=2, s=6)

        # ---------------- P1: feature planes ----------------
        sPres = sc.tile([128, JB], f32)
        sKey = sc.tile([128, JB], f32)
        sSwap = sc.tile([128, JD], f32)
        sD = sc.tile([128, JD], f32)
        sSD = sc.tile([128, JD], f32)
        sw = [sc.tile([128, JB], f32, tag=f"swp{i}", name=f"swp{i}")
              for i in range(6)]
        sT0 = sc.tile([128, JB], f32)
        sT1 = sc.tile([128, JB], f32)

        nc.vector.tensor_tensor(sT0[:], raw6[:, :, 0], raw6[:, :, 1], A.add)
        nc.vector.tensor_tensor(sT1[:], raw6[:, :, 2], raw6[:, :, 3], A.add)
        nc.vector.tensor_tensor(sT0[:], sT0[:], sT1[:], A.add)
        nc.vector.tensor_scalar(sPres[:], sT0[:], 0.0, None, A.not_equal)
        # key = cat - 1000*pres  (order-equivalent to cat + 1000*(1-pres))
        nc.vector.scalar_tensor_tensor(sKey[:], sPres[:], -1000.0,
                                       raw6[:, :, 4], A.mult, A.add)
        sKeyp = sKey.rearrange("p (m g) -> p m g", g=2)
        nc.vector.tensor_tensor(sSwap[:], sKeyp[:, :, 1], sKeyp[:, :, 0], A.is_lt)

        # compare-and-swap each of the 6 raw components + presence
        for i in range(6):
            ve, vo = rawp[:, :, 0, i], rawp[:, :, 1, i]
            dst = sw[i].rearrange("p (m g) -> p m g", g=2)
            nc.vector.tensor_tensor(sD[:], vo, ve, A.subtract)
            nc.vector.tensor_tensor(sSD[:], sD[:], sSwap[:], A.mult)
            nc.vector.tensor_tensor(dst[:, :, 0], ve, sSD[:], A.add)
            nc.vector.tensor_tensor(dst[:, :, 1], vo, sSD[:], A.subtract)
        sPresP = sPres.rearrange("p (m g) -> p m g", g=2)
        nc.vector.tensor_tensor(sD[:], sPresP[:, :, 1], sPresP[:, :, 0], A.subtract)
        nc.vector.tensor_tensor(sSD[:], sD[:], sSwap[:], A.mult)
        nc.vector.tensor_tensor(TFbp[:, :, 0, 14], sPresP[:, :, 0], sSD[:], A.add)
        nc.vector.tensor_tensor(TFbp[:, :, 1, 14], sPresP[:, :, 1], sSD[:], A.subtract)

        sX1, sY1, sX2, sY2, sCat, sConf = sw
        # normalized coords in f32 scratch (reused for derived features)
        sx1n = sc.tile([128, JB], f32)
        sy1n = sc.tile([128, JB], f32)
        sx2n = sc.tile([128, JB], f32)
        sy2n = sc.tile([128, JB], f32)
        sWf = sc.tile([128, JB], f32)
        sHf = sc.tile([128, JB], f32)
        sCX = sc.tile([128, JB], f32)
        sCY = sc.tile([128, JB], f32)
        nc.vector.tensor_scalar(sx1n[:], sX1[:], 1.0 / IW, None, A.mult)
        nc.vector.tensor_scalar(sy1n[:], sY1[:], 1.0 / IH, None, A.mult)
        nc.vector.tensor_scalar(sx2n[:], sX2[:], 1.0 / IW, None, A.mult)
        nc.vector.tensor_scalar(sy2n[:], sY2[:], 1.0 / IH, None, A.mult)
        # f0..f3 normalized coords (bf16 casts)
        nc.vector.tensor_copy(TFb[:, :, 0], sx1n[:])
        nc.vector.tensor_copy(TFb[:, :, 1], sy1n[:])
        nc.vector.tensor_copy(TFb[:, :, 2], sx2n[:])
        nc.vector.tensor_copy(TFb[:, :, 3], sy2n[:])
        # f4 w, f5 h, f6 cx*2, f7 cy*2 (the 0.5 is folded into the weights)
        nc.vector.tensor_tensor(sWf[:], sx2n[:], sx1n[:], A.subtract)
        nc.vector.tensor_tensor(sHf[:], sy2n[:], sy1n[:], A.subtract)
        nc.vector.tensor_tensor(sCX[:], sx1n[:], sx2n[:], A.add)
        nc.vector.tensor_tensor(sCY[:], sy1n[:], sy2n[:], A.add)
        nc.vector.tensor_copy(TFb[:, :, 4], sWf[:])
        nc.vector.tensor_copy(TFb[:, :, 5], sHf[:])
        nc.vector.tensor_copy(TFb[:, :, 6], sCX[:])
        nc.vector.tensor_copy(TFb[:, :, 7], sCY[:])
        # f8 area, f9 aspect = w / (h + 1e-6)
        nc.vector.tensor_tensor(TFb[:, :, 8], sWf[:], sHf[:], A.mult)
        nc.vector.tensor_scalar(sT0[:], sHf[:], 1e-6, None, A.add)
        nc.vector.reciprocal(sT1[:], sT0[:])
        nc.vector.tensor_tensor(TFb[:, :, 9], sWf[:], sT1[:], A.mult)
        # f10..12 cat one-hots * pres ; f13 conf*pres ; f15 = 1-pres
        for k in range(3):
            nc.vector.scalar_tensor_tensor(TFb[:, :, 10 + k], sCat[:], float(k),
                                           TFb[:, :, 14], A.is_equal, A.mult)
        nc.vector.tensor_tensor(TFb[:, :, 13], sConf[:], TFb[:, :, 14], A.mult)
        nc.vector.tensor_scalar(TFb[:, :, 15], TFb[:, :, 14], -1.0, 1.0,
                                A.mult, A.add)
        # dist tokens: f16 = 0.5*sqrt(dx2^2+dy2^2) (cx stored doubled), f17 = 1
        sDx = sc.tile([128, JD], f32)
        sDy = sc.tile([128, JD], f32)
        sCXp = sCX.rearrange("p (m g) -> p m g", g=2)
        sCYp = sCY.rearrange("p (m g) -> p m g", g=2)
        nc.vector.tensor_tensor(sDx[:], sCXp[:, :, 0], sCXp[:, :, 1], A.subtract)
        nc.vector.tensor_tensor(sDy[:], sCYp[:, :, 0], sCYp[:, :, 1], A.subtract)
        nc.vector.tensor_tensor(sDx[:], sDx[:], sDx[:], A.mult)
        nc.vector.tensor_tensor(sDy[:], sDy[:], sDy[:], A.mult)
        nc.vector.tensor_tensor(sDx[:], sDx[:], sDy[:], A.add)
        sDist = sc.tile([128, JD], f32)
        nc.scalar.activation(sDist[:], sDx[:], AF.Sqrt, scale=0.25)
        nc.vector.tensor_copy(TFd[:, :, 16], sDist[:])
        nc.vector.memset(TFd[:, :, 17], 1.0)

        # persistent PSUM block tiles: multiple logical slots packed per bank
        # (subtile dep tracking gives per-slot semaphores) so the PE can run
        # several iterations ahead of its consumers
        opairs = [pp.tile([128, 2 * D], f32, tag=f"op{i}", name=f"op{i}")
                  for i in range(2)]                                     # 4 banks
        zs = [pp.tile([128, DH], f32, tag=f"z{i}", name=f"z{i}")
              for i in range(2)]                                         # 2 banks
        hts_ps = [pp.tile([128, DH], bf16, tag=f"ht{i}", name=f"ht{i}")
                  for i in range(2)]                                     # 2 banks

        def zsl(k):
            return zs[k % 2][:]

        def htsl(k):
            return hts_ps[k % 2][:]

        def osl(k):
            return opairs[(k // 2) % 2][:, (k % 2) * D:(k % 2 + 1) * D]

        # ---------------- P2: transpose T_feat chunks -> bf16 lhsT tiles ----
        cta = cp.tile([128, NCHUNK * 128], bf16)
        for ci in range(NCHUNK):
            w_cols = min(128, J * F - ci * 128)
            ps = htsl(ci)[:, 0:128]
            nc.tensor.transpose(ps[:w_cols, :], TF[:, ci * 128: ci * 128 + w_cols],
                                idb)
            dst = cta[:w_cols, ci * 128: ci * 128 + 128]
            if ci % 2 == 0:
                nc.vector.tensor_copy(dst, ps[:w_cols, :])
            else:
                nc.scalar.copy(dst, ps[:w_cols, :])

        def lhsT(j):
            ci, jj = j // 4, j % 4
            return cta[32 * jj: 32 * jj + 32, ci * 128: (ci + 1) * 128]

        # ---------------- P3: LN stats via Gram trick ----------------
        VAR = cp.tile([128, NG * 4], f32)
        MU = cp.tile([128, NG * 4], f32)
        tmps = [sc.tile([128, 128], f32, tag=f"tmp{i}", name=f"tmp{i}")
                for i in range(2)]
        for g in range(NG):
            y = zsl(g)[:, 0:132]
            nc.tensor.matmul(y[:], cta[:, g * 128:(g + 1) * 128], gblk,
                             start=True, stop=True)
            # full-width contiguous ops (pad feature cols are zero, so they
            # contribute nothing to the reduction)
            tmp = tmps[g % 2]
            nc.vector.tensor_tensor(tmp[:], TF[:, g * 128:(g + 1) * 128],
                                    y[:, 0:128], A.mult)
            nc.vector.tensor_reduce(VAR[:, 4 * g: 4 * g + 4],
                                    tmp.rearrange("p (s f) -> p s f", f=32),
                                    AX.X, A.add)
            nc.scalar.copy(MU[:, 4 * g: 4 * g + 4], y[:, 128:132])

        # ---------------- P3b: batched rstd (single Sqrt region) ------------
        sdv = cp.tile([128, NG * 4], f32)
        eps = cp.tile([128, 1], f32)
        nc.vector.memset(eps[:], 1e-5)
        nc.vector.tensor_tensor(sdv[:], MU[:], MU[:], A.mult)
        nc.vector.tensor_tensor(VAR[:], VAR[:], sdv[:], A.subtract)
        nc.scalar.activation(sdv[:], VAR[:], AF.Sqrt, bias=eps[:])
        rstd = cp.tile([128, NG * 4], f32)
        nc.vector.reciprocal(rstd[:], sdv[:])
        nmr = cp.tile([128, NG * 4], f32)
        nc.vector.scalar_tensor_tensor(nmr[:], MU[:], -1.0, rstd[:],
                                       A.mult, A.mult)

        # ---------------- P5: pipelined box tiles + interleaved dist --------
        vd = out_d[:, 0:600, :].rearrange("b (q r) d -> b q r d", q=8)
        vb = out_d[:, 600:1800, :].rearrange("b (q r) d -> b q r d", q=8)
        h_tiles, hts_tiles = {}, {}
        state = {"box_stage": None, "stage_fill": 0, "box_dma": 0,
                 "dist_stage": None, "dist_fill": 0, "dist_dma": 0}
        GB, GD = 16, 8   # box / dist tiles per output stage
        dma_engs = [nc.sync, nc.gpsimd, nc.scalar]

        def emit_cast(p):
            # paired PSUM->bf16 cast for box tiles (2p, 2p+1), one iteration
            # after the pair's accumulation completed (no head-of-line wait)
            if state["box_stage"] is None:
                state["box_stage"] = bstg.tile([128, GB * D], bf16, tag="bstage",
                                               name="box_stage")
                state["stage_fill"] = 0
            sf = state["stage_fill"]
            dstc = state["box_stage"][:, sf * D:(sf + 2) * D]
            src = opairs[p % 2][:]
            if p % 2 == 0:
                nc.vector.tensor_copy(dstc, src)
            else:
                nc.scalar.copy(dstc, src)
            state["stage_fill"] += 2
            if state["stage_fill"] == GB or p == JB // 2 - 1:
                j0 = 2 * p - state["stage_fill"] + 2
                eng = dma_engs[state["box_dma"] % 3]
                eng.dma_start(vb[:, :, j0:j0 + state["stage_fill"], :],
                              state["box_stage"][:, : state["stage_fill"] * D])
                state["box_dma"] += 1
                state["box_stage"] = None

        def emit_dist(d):
            # dist tokens are rank-1 (dist*w + b): one 4x-tier DVE
            # tensor_scalar straight into SBUF staging -- no PE, no PSUM
            if state["dist_stage"] is None:
                state["dist_stage"] = dstg.tile([128, GD * D], bf16, tag="dstage",
                                                name="dist_stage")
                state["dist_fill"] = 0
            df = state["dist_fill"]
            ddst = state["dist_stage"][:, df * D:(df + 1) * D]
            nc.vector.tensor_scalar(ddst, wrep, sDist[:, d:d + 1],
                                    None, A.mult)
            if not dist_b_zero:
                nc.vector.tensor_tensor(ddst, ddst, brep, A.add)
            state["dist_fill"] += 1
            if state["dist_fill"] == GD or d == JD - 1:
                d0 = d - state["dist_fill"] + 1
                eng = dma_engs[(state["dist_dma"] + 1) % 3]
                eng.dma_start(vd[:, :, d0:d0 + state["dist_fill"], :],
                              state["dist_stage"][:, : state["dist_fill"] * D])
                state["dist_dma"] += 1
                state["dist_stage"] = None

        for k in range(-3, JB + 2):
            kz, kh, kt, kc, ko = k + 3, k + 2, k + 1, k + 1, k
            if 0 <= kz < JB:
                jj = kz % 4
                nc.tensor.matmul(zsl(kz), lhsT(kz), w1[32 * jj: 32 * jj + 32, :],
                                 start=True, stop=True,
                                 tile_position=(32 * jj, 0))
            if 0 <= kh < JB:
                h = hp.tile([128, DH], bf16, tag="h", name="h")
                nc.scalar.activation(h[:], zsl(kh), AF.Gelu,
                                     bias=nmr[:, kh:kh + 1],
                                     scale=rstd[:, kh:kh + 1])
                h_tiles[kh] = h
            if 0 <= kt < JB:
                ht = htsl(kt)
                hin = h_tiles.pop(kt)
                nc.tensor.transpose(ht[:, 0:128], hin[:, 0:128], idb)
                nc.tensor.transpose(ht[:, 128:256], hin[:, 128:256], idb)
            if 0 <= kc < JB:
                hts = tsp.tile([128, DH], bf16, tag="hts", name="hts")
                nc.vector.tensor_copy(hts[:], htsl(kc))
                hts_tiles[kc] = hts
            if 0 <= ko < JB:
                jj = ko % 4
                cam = (ko % 6) // 2
                hs = hts_tiles.pop(ko)
                o = osl(ko)
                nc.tensor.matmul(o, hs[:, 0:128], w2hi, start=True, stop=False)
                nc.tensor.matmul(o, hs[:, 128:256], w2lo, start=False, stop=False)
                nc.tensor.matmul(o, lhsT(ko), w2x[cam][32 * jj: 32 * jj + 32, :],
                                 start=False, stop=True,
                                 tile_position=(32 * jj, 0))
                if ko % 2 == 0 and ko // 2 < JD:
                    emit_dist(ko // 2)
            if 2 <= ko <= JB and ko % 2 == 0:
                emit_cast((ko - 2) // 2)

    nc.compile()
    return nc


def _prep_inputs(inputs):
    f32 = np.float32
    bf = ml_dtypes.bfloat16
    scale = float(np.asarray(inputs["scale"]))

    W1p = np.zeros((32, DH), f32)
    W1p[0:10] = np.asarray(inputs["geom_w1"], f32)
    W1p[6] *= 0.5
    W1p[7] *= 0.5
    w1rep = np.tile(W1p, (4, 1))

    W2s = scale * np.asarray(inputs["geom_w2"], f32)
    w2hi, w2lo = W2s[:128], W2s[128:]

    cat_t = np.asarray(inputs["cat_table"], f32)
    cam_t = np.asarray(inputs["cam_table"], f32)
    bias_row = (np.asarray(inputs["geom_b2"], f32)
                + np.asarray(inputs["conf_b"], f32)
                + np.asarray(inputs["center_b"], f32))
    w2x_reps = []
    for c in range(3):
        W2x = np.zeros((32, D), f32)
        W2x[6] = scale * np.asarray(inputs["center_w"], f32)[0] * 0.5
        W2x[7] = scale * np.asarray(inputs["center_w"], f32)[1] * 0.5
        W2x[10:13] = scale * cat_t
        W2x[13] = scale * np.asarray(inputs["conf_w"], f32)[0]
        W2x[14] = scale * (bias_row + cam_t[c])
        W2x[15] = np.asarray(inputs["missing_emb"], f32)[0]
        W2x[16] = np.asarray(inputs["dist_w"], f32)[0]
        W2x[17] = np.asarray(inputs["dist_b"], f32)
        w2x_reps.append(np.tile(W2x, (4, 1)))

    G = (W1p @ W1p.T) / 256.0
    w_mu = W1p.sum(axis=1) / 256.0
    gblk = np.zeros((128, 132), f32)
    for s in range(4):
        gblk[32 * s:32 * s + 32, 32 * s:32 * s + 32] = G
        gblk[32 * s:32 * s + 32, 128 + s] = w_mu

    idb = np.eye(128, dtype=f32)
    wrep = np.tile(np.asarray(inputs["dist_w"], f32), (128, 1))
    brep = np.tile(np.asarray(inputs["dist_b"], f32)[None, :], (128, 1))
    bpk = np.concatenate(
        [idb, w1rep, w2hi, w2lo] + w2x_reps + [gblk, wrep, brep], axis=1
    ).astype(bf)

    box = np.asarray(inputs["box_data"], f32)
    fpks = []
    for c in range(NCORES):
        rawc = box[c * BPC:(c + 1) * BPC].reshape(BPC, T * 6, 6)
        rawc = rawc.reshape(BPC, 8, JB, 6).reshape(128, 900)
        fpks.append(np.ascontiguousarray(rawc, dtype=f32))
    return fpks, bpk


def _fast_path_ok(inputs):
    try:
        shapes = {
            "box_data": (B, T, 6, 6), "cat_table": (3, D), "geom_w1": (10, DH),
            "geom_b1": (DH,), "ln_g": (DH,), "ln_b": (DH,), "geom_w2": (DH, D),
            "geom_b2": (D,), "conf_w": (1, D), "conf_b": (D,),
            "center_w": (2, D), "center_b": (D,), "missing_emb": (1, D),
            "dist_w": (1, D), "dist_b": (D,), "cam_table": (NCAM, D),
        }
        for k, s in shapes.items():
            if tuple(np.asarray(inputs[k]).shape) != s:
                return False
        if not np.all(np.asarray(inputs["geom_b1"]) == 0):
            return False
        if not np.all(np.asarray(inputs["ln_g"]) == 1):
            return False
        if not np.all(np.asarray(inputs["ln_b"]) == 0):
            return False
        return True
    except Exception:
        return False


def _numpy_fallback(inputs):
    # Exact (slow) port of the reference for unexpected inputs.
    import math
    f32 = np.float32
    inp = {k: np.asarray(v) for k, v in inputs.items()}
    coords = inp["box_data"][..., :4].astype(f32)
    category = inp["box_data"][..., 4].astype(np.int32)
    conf = inp["box_data"][..., 5].astype(f32)
    norm = np.array([IW, IH, IW, IH], f32)
    cn = (coords / norm).reshape(B, T, NCAM, NB, 4)
    category = category.reshape(B, T, NCAM, NB)
    conf = conf.reshape(B, T, NCAM, NB, 1)
    presence = (cn.sum(-1) != 0).astype(f32)
    sort_key = category.astype(f32) + (1.0 - presence) * 1000.0
    idx = np.argsort(sort_key, axis=-1, kind="stable")
    cn = np.take_along_axis(cn, idx[..., None], axis=-2)
    category = np.take_along_axis(category, idx, axis=-1)
    conf = np.take_along_axis(conf, idx[..., None], axis=-2)
    presence = (cn.sum(-1) != 0).astype(f32)[..., None]
    x1, y1, x2, y2 = cn[..., 0], cn[..., 1], cn[..., 2], cn[..., 3]
    w, h = x2 - x1, y2 - y1
    cx, cy = (x1 + x2) * 0.5, (y1 + y2) * 0.5
    area, aspect = w * h, w / (h + 1e-6)
    dx, dy = cx[..., 0] - cx[..., 1], cy[..., 0] - cy[..., 1]
    dist = np.sqrt(dx * dx + dy * dy)[..., None]
    dist_tok = dist @ inp["dist_w"].astype(f32) + inp["dist_b"].astype(f32)
    geom = np.stack([x1, y1, x2, y2, w, h, cx, cy, area, aspect], axis=-1)
    z = geom @ inp["geom_w1"].astype(f32) + inp["geom_b1"].astype(f32)
    mu = z.mean(-1, keepdims=True)
    var = ((z - mu) ** 2).mean(-1, keepdims=True)
    xh = (z - mu) / np.sqrt(var + 1e-5) * inp["ln_g"].astype(f32) + inp["ln_b"].astype(f32)
    try:
        from scipy.special import erf as _erf
        g = xh * 0.5 * (1.0 + _erf(xh / np.sqrt(2.0)))
    except Exception:
        verf = np.vectorize(math.erf)
        g = xh * 0.5 * (1.0 + verf(xh / np.sqrt(2.0)))
    geom_p = g @ inp["geom_w2"].astype(f32) + inp["geom_b2"].astype(f32)
    cat_emb = inp["cat_table"].astype(f32)[category]
    conf_p = conf @ inp["conf_w"].astype(f32) + inp["conf_b"].astype(f32)
    center_p = np.stack([cx, cy], axis=-1) @ inp["center_w"].astype(f32) + inp["center_b"].astype(f32)
    cam_emb = inp["cam_table"].astype(f32).reshape(1, 1, NCAM, 1, D)
    tok = (geom_p + cat_emb + conf_p + center_p + cam_emb) * float(inp["scale"])
    tok = np.where(presence == 0, inp["missing_emb"].astype(f32)[0], tok)
    out = np.concatenate([dist_tok.reshape(B, T * NCAM, D),
                          tok.reshape(B, T * NCAM * NB, D)], axis=1)
    return out.astype(np.float32)


def _run(inputs, trace=False, tmpdir=None):
    from concourse.bass_utils import run_bass_kernel_spmd

    dbz = bool(np.all(np.asarray(inputs["dist_b"]) == 0))
    key = ("nc", dbz)
    if key not in _CACHE:
        _CACHE[key] = _build_nc(dist_b_zero=dbz)
    nc = _CACHE[key]

    fpks, bpk = _prep_inputs(inputs)
    in_maps = [{"fpk": fpks[c], "bpk": bpk} for c in range(NCORES)]
    res = run_bass_kernel_spmd(nc, in_maps, list(range(NCORES)),
                               trace=trace, tmpdir=tmpdir)
    out = np.concatenate([np.asarray(res.results[c]["out"])
                          for c in range(NCORES)], axis=0)
    return out.astype(np.float32), res


def kernel(**inputs):
    if not _fast_path_ok(inputs):
        return _numpy_fallback(inputs)
    out, _ = _run(inputs)
    return out


if __name__ == "__main__":
    import reference as ref
    inputs = {k: np.asarray(v) for k, v in ref.setup_inputs().items()}
    got = kernel(**inputs)
    exp = np.load("/tmp/expected.npy")
    d = got - exp
    print("rel fro:", np.linalg.norm(d) / np.linalg.norm(exp))
    print("absmax rel:", np.abs(d).max() / np.abs(exp).max())


# revision 35
# speedup vs baseline: 1.0576x; 1.0308x over previous
"""Trainium2 Bass kernel for nn_BoxEncoder (B=128, T=200, NC=3, NB=2, D=512, DH=256).

Strategy (data-parallel over batch, 16 batch items per core x 8 cores):

 - The within-camera argsort over NB=2 boxes reduces to a single
   compare-and-swap (stable sort of 2 keys).
 - All per-box scalars are computed as [128, n] "feature planes" with DVE
   ops, laid out 32 feature-columns per token in a big bf16 T_feat tile
   (box slots j=0..149 first, dist slots j=150..224).
 - PE transposes of [128,128] chunks of T_feat produce feature-major lhsT
   tiles (32-aligned partition bases) feeding the matmuls.
 - LayerNorm stats via a Gram-matrix trick: var = x.(x@G) - mu^2 with
   G = W1@W1^T/256, computed by 38 block-diagonal matmuls (N=132) plus
   DVE segmented multiply-reduce -- no second z pass, no bn_stats.
 - Both Sqrt batches (dist feature + LN rstd) run before any GELU so the
   ACT spline table switches exactly once.
 - P5 is software-pipelined with stage offsets (z:+3, gelu:+2,
   transpose:+1, out:+0) so every cross-engine dependency is satisfied a
   full iteration ahead -- the PE issues back-to-back matmuls, keeping
   the HAM clock-gate warm (2.4 GHz).
 - Outputs are staged and DMA'd as bf16 (halved HBM traffic; host
   upcasts), box rows on the sync queue, dist rows on the gpsimd queue.
 - Missing boxes produce exactly missing_emb through the matmul (their
   geom path contributes gelu(0)=0), so no select/where is needed.

Token layout per core: partition p = bt*8 + q (bt = batch item 0..15,
q = 0..7). Box slot j in [0,150) covers output rows bt*1800 + 600 +
q*150 + j; dist slot j in [150,225) covers rows bt*1800 + q*75 + (j-150).
"""

import numpy as np
import ml_dtypes

B, T, NCAM, NB, D, DH = 128, 200, 3, 2, 512, 256
IW, IH = 640.0, 400.0
NCORES = 8
BPC = B // NCORES            # batch items per core
JB, JD = 150, 75             # box / dist j-slots per partition
J = JB + JD                  # 225
F = 32                       # feature columns per j-slot
NCHUNK = (J * F + 127) // 128   # 57 transpose chunks (56 full + 1 of 32 cols)
NG = (JB + 3) // 4           # 38 stats chunks (chunk 37 partly dist, harmless)

_CACHE = {}

# bf16 pack column offsets
C_ID = 0
C_W1 = C_ID + 128
C_W2HI = C_W1 + 256
C_W2LO = C_W2HI + 512
C_W2X = C_W2LO + 512          # 3 cam variants, 512 each
C_GBLK = C_W2X + 3 * 512
C_WREP = C_GBLK + 132         # dist_w replicated over partitions
C_BREP = C_WREP + 512         # dist_b replicated over partitions
NBF = C_BREP + 512


def _build_nc(dist_b_zero=True):
    from contextlib import ExitStack
    import concourse.bacc as bacc
    import concourse.mybir as mybir
    import concourse.tile as tile

    f32 = mybir.dt.float32
    bf16 = mybir.dt.bfloat16
    A = mybir.AluOpType
    AF = mybir.ActivationFunctionType
    AX = mybir.AxisListType

    nc = bacc.Bacc("TRN2", target_bir_lowering=False, debug=False,
                   num_devices=NCORES)
    fpk = nc.declare_dram_parameter("fpk", [128, 900], f32, isOutput=False)
    bpk = nc.declare_dram_parameter("bpk", [128, NBF], bf16, isOutput=False)
    out_d = nc.declare_dram_parameter("out", [BPC, 1800, D], bf16, isOutput=True)

    with ExitStack() as ctx:
        tc = ctx.enter_context(tile.TileContext(nc))
        cp = ctx.enter_context(tc.tile_pool(name="const", bufs=1))
        sc = ctx.enter_context(tc.tile_pool(name="scratch", bufs=1))
        pp = ctx.enter_context(tc.tile_pool(name="ps", bufs=1, space="PSUM"))
        hp = ctx.enter_context(tc.tile_pool(name="hsb", bufs=6))
        tsp = ctx.enter_context(tc.tile_pool(name="htsb", bufs=4))
        bstg = ctx.enter_context(tc.tile_pool(name="bstage", bufs=2))
        dstg = ctx.enter_context(tc.tile_pool(name="dstage", bufs=2))

        fpack = cp.tile([128, 900], f32)
        nc.sync.dma_start(fpack[:], fpk[:])
        bpack = cp.tile([128, NBF], bf16)
        nc.sync.dma_start(bpack[:], bpk[:])

        raw = fpack[:, 0:900]
        idb = bpack[:, C_ID:C_ID + 128]
        w1 = bpack[:, C_W1:C_W1 + 256]
        w2hi = bpack[:, C_W2HI:C_W2HI + 512]
        w2lo = bpack[:, C_W2LO:C_W2LO + 512]
        w2x = [bpack[:, C_W2X + c * 512: C_W2X + (c + 1) * 512] for c in range(3)]
        gblk = bpack[:, C_GBLK:C_GBLK + 132]
        wrep = bpack[:, C_WREP:C_WREP + 512]
        brep = bpack[:, C_BREP:C_BREP + 512]

        TF = cp.tile([128, J * F], bf16)
        nc.gpsimd.memset(TF[:], 0.0)

        TFj = TF.rearrange("p (j f) -> p j f", f=F)
        TFb = TFj[:, :JB, :]                       # box slots (first now)
        TFd = TFj[:, JB:, :]                       # dist slots
        TFbp = TF[:, : JB * F].rearrange("p (m g f) -> p m g f", g=2, f=F)
        raw6 = raw.rearrange("p (b s) -> p b s", s=6)
        rawp = raw.rearrange("p (m g s) -> p m g s", g=2, s=6)

        # ---------------- P1: feature planes ----------------
        sPres = sc.tile([128, JB], f32)
        sKey = sc.tile([128, JB], f32)
        sSwap = sc.tile([128, JD], f32)
        sD = sc.tile([128, JD], f32)
        sSD = sc.tile([128, JD], f32)
        sw = [sc.tile([128, JB], f32, tag=f"swp{i}", name=f"swp{i}")
              for i in range(6)]
        sT0 = sc.tile([128, JB], f32)
        sT1 = sc.tile([128, JB], f32)

        nc.vector.tensor_tensor(sT0[:], raw6[:, :, 0], raw6[:, :, 1], A.add)
        nc.vector.tensor_tensor(sT1[:], raw6[:, :, 2], raw6[:, :, 3], A.add)
        nc.vector.tensor_tensor(sT0[:], sT0[:], sT1[:], A.add)
        nc.vector.tensor_scalar(sPres[:], sT0[:], 0.0, None, A.not_equal)
        # key = cat - 1000*pres  (order-equivalent to cat + 1000*(1-pres))
        nc.vector.scalar_tensor_tensor(sKey[:], sPres[:], -1000.0,
                                       raw6[:, :, 4], A.mult, A.add)
        sKeyp = sKey.rearrange("p (m g) -> p m g", g=2)
        nc.vector.tensor_tensor(sSwap[:], sKeyp[:, :, 1], sKeyp[:, :, 0], A.is_lt)

        # compare-and-swap each of the 6 raw components + presence
        for i in range(6):
            ve, vo = rawp[:, :, 0, i], rawp[:, :, 1, i]
            dst = sw[i].rearrange("p (m g) -> p m g", g=2)
            nc.vector.tensor_tensor(sD[:], vo, ve, A.subtract)
            nc.vector.tensor_tensor(sSD[:], sD[:], sSwap[:], A.mult)
            nc.vector.tensor_tensor(dst[:, :, 0], ve, sSD[:], A.add)
            nc.vector.tensor_tensor(dst[:, :, 1], vo, sSD[:], A.subtract)
        sPresP = sPres.rearrange("p (m g) -> p m g", g=2)
        nc.vector.tensor_tensor(sD[:], sPresP[:, :, 1], sPresP[:, :, 0], A.subtract)
        nc.vector.tensor_tensor(sSD[:], sD[:], sSwap[:], A.mult)
        nc.vector.tensor_tensor(TFbp[:, :, 0, 14], sPresP[:, :, 0], sSD[:], A.add)
        nc.vector.tensor_tensor(TFbp[:, :, 1, 14], sPresP[:, :, 1], sSD[:], A.subtract)

        sX1, sY1, sX2, sY2, sCat, sConf = sw
        # normalized coords in f32 scratch (reused for derived features)
        sx1n = sc.tile([128, JB], f32)
        sy1n = sc.tile([128, JB], f32)
        sx2n = sc.tile([128, JB], f32)
        sy2n = sc.tile([128, JB], f32)
        sWf = sc.tile([128, JB], f32)
        sHf = sc.tile([128, JB], f32)
        sCX = sc.tile([128, JB], f32)
        sCY = sc.tile([128, JB], f32)
        nc.vector.tensor_scalar(sx1n[:], sX1[:], 1.0 / IW, None, A.mult)
        nc.vector.tensor_scalar(sy1n[:], sY1[:], 1.0 / IH, None, A.mult)
        nc.vector.tensor_scalar(sx2n[:], sX2[:], 1.0 / IW, None, A.mult)
        nc.vector.tensor_scalar(sy2n[:], sY2[:], 1.0 / IH, None, A.mult)
        # f0..f3 normalized coords (bf16 casts)
        nc.vector.tensor_copy(TFb[:, :, 0], sx1n[:])
        nc.vector.tensor_copy(TFb[:, :, 1], sy1n[:])
        nc.vector.tensor_copy(TFb[:, :, 2], sx2n[:])
        nc.vector.tensor_copy(TFb[:, :, 3], sy2n[:])
        # f4 w, f5 h, f6 cx*2, f7 cy*2 (the 0.5 is folded into the weights)
        nc.vector.tensor_tensor(sWf[:], sx2n[:], sx1n[:], A.subtract)
        nc.vector.tensor_tensor(sHf[:], sy2n[:], sy1n[:], A.subtract)
        nc.vector.tensor_tensor(sCX[:], sx1n[:], sx2n[:], A.add)
        nc.vector.tensor_tensor(sCY[:], sy1n[:], sy2n[:], A.add)
        nc.vector.tensor_copy(TFb[:, :, 4], sWf[:])
        nc.vector.tensor_copy(TFb[:, :, 5], sHf[:])
        nc.vector.tensor_copy(TFb[:, :, 6], sCX[:])
        nc.vector.tensor_copy(TFb[:, :, 7], sCY[:])
        # f8 area, f9 aspect = w / (h + 1e-6)
        nc.vector.tensor_tensor(TFb[:, :, 8], sWf[:], sHf[:], A.mult)
        nc.vector.tensor_scalar(sT0[:], sHf[:], 1e-6, None, A.add)
        nc.vector.reciprocal(sT1[:], sT0[:])
        nc.vector.tensor_tensor(TFb[:, :, 9], sWf[:], sT1[:], A.mult)
        # f10..12 cat one-hots * pres ; f13 conf*pres ; f15 = 1-pres
        for k in range(3):
            nc.vector.scalar_tensor_tensor(TFb[:, :, 10 + k], sCat[:], float(k),
                                           TFb[:, :, 14], A.is_equal, A.mult)
        nc.vector.tensor_tensor(TFb[:, :, 13], sConf[:], TFb[:, :, 14], A.mult)
        nc.vector.tensor_scalar(TFb[:, :, 15], TFb[:, :, 14], -1.0, 1.0,
                                A.mult, A.add)
        # dist tokens: f16 = 0.5*sqrt(dx2^2+dy2^2) (cx stored doubled), f17 = 1
        sDx = sc.tile([128, JD], f32)
        sDy = sc.tile([128, JD], f32)
        sCXp = sCX.rearrange("p (m g) -> p m g", g=2)
        sCYp = sCY.rearrange("p (m g) -> p m g", g=2)
        nc.vector.tensor_tensor(sDx[:], sCXp[:, :, 0], sCXp[:, :, 1], A.subtract)
        nc.vector.tensor_tensor(sDy[:], sCYp[:, :, 0], sCYp[:, :, 1], A.subtract)
        nc.vector.tensor_tensor(sDx[:], sDx[:], sDx[:], A.mult)
        nc.vector.tensor_tensor(sDy[:], sDy[:], sDy[:], A.mult)
        nc.vector.tensor_tensor(sDx[:], sDx[:], sDy[:], A.add)
        sDist = sc.tile([128, JD], f32)
        nc.scalar.activation(sDist[:], sDx[:], AF.Sqrt, scale=0.25)
        nc.vector.tensor_copy(TFd[:, :, 16], sDist[:])
        nc.vector.memset(TFd[:, :, 17], 1.0)

        # persistent PSUM block tiles: multiple logical slots packed per bank
        # (subtile dep tracking gives per-slot semaphores) so the PE can run
        # several iterations ahead of its consumers
        opairs = [pp.tile([128, 2 * D], f32, tag=f"op{i}", name=f"op{i}")
                  for i in range(2)]                                     # 4 banks
        zs = [pp.tile([128, DH], f32, tag=f"z{i}", name=f"z{i}")
              for i in range(2)]                                         # 2 banks
        hts_ps = [pp.tile([128, DH], bf16, tag=f"ht{i}", name=f"ht{i}")
                  for i in range(2)]                                     # 2 banks

        def zsl(k):
            return zs[k % 2][:]

        def htsl(k):
            return hts_ps[k % 2][:]

        def osl(k):
            return opairs[(k // 2) % 2][:, (k % 2) * D:(k % 2 + 1) * D]

        # ---------------- P2: transpose T_feat chunks -> bf16 lhsT tiles ----
        cta = cp.tile([128, NCHUNK * 128], bf16)
        for ci in range(NCHUNK):
            w_cols = min(128, J * F - ci * 128)
            ps = htsl(ci)[:, 0:128]
            nc.tensor.transpose(ps[:w_cols, :], TF[:, ci * 128: ci * 128 + w_cols],
                                idb)
            dst = cta[:w_cols, ci * 128: ci * 128 + 128]
            if ci % 2 == 0:
                nc.vector.tensor_copy(dst, ps[:w_cols, :])
            else:
                nc.scalar.copy(dst, ps[:w_cols, :])

        def lhsT(j):
            ci, jj = j // 4, j % 4
            return cta[32 * jj: 32 * jj + 32, ci * 128: (ci + 1) * 128]

        # ---------------- P3: LN stats via Gram trick ----------------
        VAR = cp.tile([128, NG * 4], f32)
        MU = cp.tile([128, NG * 4], f32)
        tmps = [sc.tile([128, 128], f32, tag=f"tmp{i}", name=f"tmp{i}")
                for i in range(2)]
        for g in range(NG):
            y = zsl(g)[:, 0:132]
            nc.tensor.matmul(y[:], cta[:, g * 128:(g + 1) * 128], gblk,
                             start=True, stop=True)
            # full-width contiguous ops (pad feature cols are zero, so they
            # contribute nothing to the reduction)
            tmp = tmps[g % 2]
            nc.vector.tensor_tensor(tmp[:], TF[:, g * 128:(g + 1) * 128],
                                    y[:, 0:128], A.mult)
            nc.vector.tensor_reduce(VAR[:, 4 * g: 4 * g + 4],
                                    tmp.rearrange("p (s f) -> p s f", f=32),
                                    AX.X, A.add)
            nc.scalar.copy(MU[:, 4 * g: 4 * g + 4], y[:, 128:132])

        # ---------------- P3b: batched rstd (single Sqrt region) ------------
        sdv = cp.tile([128, NG * 4], f32)
        eps = cp.tile([128, 1], f32)
        nc.vector.memset(eps[:], 1e-5)
        nc.vector.tensor_tensor(sdv[:], MU[:], MU[:], A.mult)
        nc.vector.tensor_tensor(VAR[:], VAR[:], sdv[:], A.subtract)
        nc.scalar.activation(sdv[:], VAR[:], AF.Sqrt, bias=eps[:])
        rstd = cp.tile([128, NG * 4], f32)
        nc.vector.reciprocal(rstd[:], sdv[:])
        nmr = cp.tile([128, NG * 4], f32)
        nc.vector.scalar_tensor_tensor(nmr[:], MU[:], -1.0, rstd[:],
                                       A.mult, A.mult)

        # ---------------- P5: pipelined box tiles + interleaved dist --------
        vd = out_d[:, 0:600, :].rearrange("b (q r) d -> b q r d", q=8)
        vb = out_d[:, 600:1800, :].rearrange("b (q r) d -> b q r d", q=8)
        h_tiles, hts_tiles = {}, {}
        state = {"box_stage": None, "stage_fill": 0, "box_dma": 0,
                 "dist_stage": None, "dist_fill": 0, "dist_dma": 0}
        GB, GD = 16, 8   # box / dist tiles per output stage
        dma_engs = [nc.sync, nc.gpsimd, nc.scalar]

        def emit_cast(p):
            # paired PSUM->bf16 cast for box tiles (2p, 2p+1), one iteration
            # after the pair's accumulation completed (no head-of-line wait)
            if state["box_stage"] is None:
                state["box_stage"] = bstg.tile([128, GB * D], bf16, tag="bstage",
                                               name="box_stage")
                state["stage_fill"] = 0
            sf = state["stage_fill"]
            dstc = state["box_stage"][:, sf * D:(sf + 2) * D]
            src = opairs[p % 2][:]
            if p % 2 == 0:
                nc.vector.tensor_copy(dstc, src)
            else:
                nc.scalar.copy(dstc, src)
            state["stage_fill"] += 2
            if state["stage_fill"] == GB or p == JB // 2 - 1:
                j0 = 2 * p - state["stage_fill"] + 2
                eng = dma_engs[state["box_dma"] % 3]
                eng.dma_start(vb[:, :, j0:j0 + state["stage_fill"], :],
                              state["box_stage"][:, : state["stage_fill"] * D])
                state["box_dma"] += 1
                state["box_stage"] = None

        def emit_dist(d):
            # dist tokens are rank-1 (dist*w + b): one 4x-tier DVE
            # tensor_scalar straight into SBUF staging -- no PE, no PSUM
            if state["dist_stage"] is None:
                state["dist_stage"] = dstg.tile([128, GD * D], bf16, tag="dstage",
                                                name="dist_stage")
                state["dist_fill"] = 0
            df = state["dist_fill"]
            ddst = state["dist_stage"][:, df * D:(df + 1) * D]
            nc.vector.tensor_scalar(ddst, wrep, sDist[:, d:d + 1],
                                    None, A.mult)
            if not dist_b_zero:
                nc.vector.tensor_tensor(ddst, ddst, brep, A.add)
            state["dist_fill"] += 1
            if state["dist_fill"] == GD or d == JD - 1:
                d0 = d - state["dist_fill"] + 1
                eng = dma_engs[(state["dist_dma"] + 1) % 3]
                eng.dma_start(vd[:, :, d0:d0 + state["dist_fill"], :],
                              state["dist_stage"][:, : state["dist_fill"] * D])
                state["dist_dma"] += 1
                state["dist_stage"] = None

        for k in range(-5, JB + 2):
            kz, kh, kt, kc, ko = k + 5, k + 4, k + 1, k + 1, k
            if 0 <= kz < JB:
                jj = kz % 4
                nc.tensor.matmul(zsl(kz), lhsT(kz), w1[32 * jj: 32 * jj + 32, :],
                                 start=True, stop=True,
                                 tile_position=(32 * jj, 0))
            if 0 <= kh < JB:
                h = hp.tile([128, DH], bf16, tag="h", name="h")
                nc.scalar.activation(h[:], zsl(kh), AF.Gelu,
                                     bias=nmr[:, kh:kh + 1],
                                     scale=rstd[:, kh:kh + 1])
                h_tiles[kh] = h
            if 0 <= kt < JB:
                ht = htsl(kt)
                hin = h_tiles.pop(kt)
                nc.tensor.transpose(ht[:, 0:128], hin[:, 0:128], idb)
                nc.tensor.transpose(ht[:, 128:256], hin[:, 128:256], idb)
            if 0 <= kc < JB:
                hts = tsp.tile([128, DH], bf16, tag="hts", name="hts")
                nc.vector.tensor_copy(hts[:], htsl(kc))
                hts_tiles[kc] = hts
            if 0 <= ko < JB:
                jj = ko % 4
                cam = (ko % 6) // 2
                hs = hts_tiles.pop(ko)
                o = osl(ko)
                nc.tensor.matmul(o, hs[:, 0:128], w2hi, start=True, stop=False)
                nc.tensor.matmul(o, hs[:, 128:256], w2lo, start=False, stop=False)
                nc.tensor.matmul(o, lhsT(ko), w2x[cam][32 * jj: 32 * jj + 32, :],
                                 start=False, stop=True,
                                 tile_position=(32 * jj, 0))
                if ko % 2 == 0 and ko // 2 < JD:
                    emit_dist(ko // 2)
            if 2 <= ko <= JB and ko % 2 == 0:
                emit_cast((ko - 2) // 2)

    nc.compile()
    return nc


def _prep_inputs(inputs):
    f32 = np.float32
    bf = ml_dtypes.bfloat16
    scale = float(np.asarray(inputs["scale"]))

    W1p = np.zeros((32, DH), f32)
    W1p[0:10] = np.asarray(inputs["geom_w1"], f32)
    W1p[6] *= 0.5
    W1p[7] *= 0.5
    w1rep = np.tile(W1p, (4, 1))

    W2s = scale * np.asarray(inputs["geom_w2"], f32)
    w2hi, w2lo = W2s[:128], W2s[128:]

    cat_t = np.asarray(inputs["cat_table"], f32)
    cam_t = np.asarray(inputs["cam_table"], f32)
    bias_row = (np.asarray(inputs["geom_b2"], f32)
                + np.asarray(inputs["conf_b"], f32)
                + np.asarray(inputs["center_b"], f32))
    w2x_reps = []
    for c in range(3):
        W2x = np.zeros((32, D), f32)
        W2x[6] = scale * np.asarray(inputs["center_w"], f32)[0] * 0.5
        W2x[7] = scale * np.asarray(inputs["center_w"], f32)[1] * 0.5
        W2x[10:13] = scale * cat_t
        W2x[13] = scale * np.asarray(inputs["conf_w"], f32)[0]
        W2x[14] = scale * (bias_row + cam_t[c])
        W2x[15] = np.asarray(inputs["missing_emb"], f32)[0]
        W2x[16] = np.asarray(inputs["dist_w"], f32)[0]
        W2x[17] = np.asarray(inputs["dist_b"], f32)
        w2x_reps.append(np.tile(W2x, (4, 1)))

    G = (W1p @ W1p.T) / 256.0
    w_mu = W1p.sum(axis=1) / 256.0
    gblk = np.zeros((128, 132), f32)
    for s in range(4):
        gblk[32 * s:32 * s + 32, 32 * s:32 * s + 32] = G
        gblk[32 * s:32 * s + 32, 128 + s] = w_mu

    idb = np.eye(128, dtype=f32)
    wrep = np.tile(np.asarray(inputs["dist_w"], f32), (128, 1))
    brep = np.tile(np.asarray(inputs["dist_b"], f32)[None, :], (128, 1))
    bpk = np.concatenate(
        [idb, w1rep, w2hi, w2lo] + w2x_reps + [gblk, wrep, brep], axis=1
    ).astype(bf)

    box = np.asarray(inputs["box_data"], f32)
    fpks = []
    for c in range(NCORES):
        rawc = box[c * BPC:(c + 1) * BPC].reshape(BPC, T * 6, 6)
        rawc = rawc.reshape(BPC, 8, JB, 6).reshape(128, 900)
        fpks.append(np.ascontiguousarray(rawc, dtype=f32))
    return fpks, bpk


def _fast_path_ok(inputs):
    try:
        shapes = {
            "box_data": (B, T, 6, 6), "cat_table": (3, D), "geom_w1": (10, DH),
            "geom_b1": (DH,), "ln_g": (DH,), "ln_b": (DH,), "geom_w2": (DH, D),
            "geom_b2": (D,), "conf_w": (1, D), "conf_b": (D,),
            "center_w": (2, D), "center_b": (D,), "missing_emb": (1, D),
            "dist_w": (1, D), "dist_b": (D,), "cam_table": (NCAM, D),
        }
        for k, s in shapes.items():
            if tuple(np.asarray(inputs[k]).shape) != s:
                return False
        if not np.all(np.asarray(inputs["geom_b1"]) == 0):
            return False
        if not np.all(np.asarray(inputs["ln_g"]) == 1):
            return False
        if not np.all(np.asarray(inputs["ln_b"]) == 0):
            return False
        return True
    except Exception:
        return False


def _numpy_fallback(inputs):
    # Exact (slow) port of the reference for unexpected inputs.
    import math
    f32 = np.float32
    inp = {k: np.asarray(v) for k, v in inputs.items()}
    coords = inp["box_data"][..., :4].astype(f32)
    category = inp["box_data"][..., 4].astype(np.int32)
    conf = inp["box_data"][..., 5].astype(f32)
    norm = np.array([IW, IH, IW, IH], f32)
    cn = (coords / norm).reshape(B, T, NCAM, NB, 4)
    category = category.reshape(B, T, NCAM, NB)
    conf = conf.reshape(B, T, NCAM, NB, 1)
    presence = (cn.sum(-1) != 0).astype(f32)
    sort_key = category.astype(f32) + (1.0 - presence) * 1000.0
    idx = np.argsort(sort_key, axis=-1, kind="stable")
    cn = np.take_along_axis(cn, idx[..., None], axis=-2)
    category = np.take_along_axis(category, idx, axis=-1)
    conf = np.take_along_axis(conf, idx[..., None], axis=-2)
    presence = (cn.sum(-1) != 0).astype(f32)[..., None]
    x1, y1, x2, y2 = cn[..., 0], cn[..., 1], cn[..., 2], cn[..., 3]
    w, h = x2 - x1, y2 - y1
    cx, cy = (x1 + x2) * 0.5, (y1 + y2) * 0.5
    area, aspect = w * h, w / (h + 1e-6)
    dx, dy = cx[..., 0] - cx[..., 1], cy[..., 0] - cy[..., 1]
    dist = np.sqrt(dx * dx + dy * dy)[..., None]
    dist_tok = dist @ inp["dist_w"].astype(f32) + inp["dist_b"].astype(f32)
    geom = np.stack([x1, y1, x2, y2, w, h, cx, cy, area, aspect], axis=-1)
    z = geom @ inp["geom_w1"].astype(f32) + inp["geom_b1"].astype(f32)
    mu = z.mean(-1, keepdims=True)
    var = ((z - mu) ** 2).mean(-1, keepdims=True)
    xh = (z - mu) / np.sqrt(var + 1e-5) * inp["ln_g"].astype(f32) + inp["ln_b"].astype(f32)
    try:
        from scipy.special import erf as _erf
        g = xh * 0.5 * (1.0 + _erf(xh / np.sqrt(2.0)))
    except Exception:
        verf = np.vectorize(math.erf)
        g = xh * 0.5 * (1.0 + verf(xh / np.sqrt(2.0)))
    geom_p = g @ inp["geom_w2"].astype(f32) + inp["geom_b2"].astype(f32)
    cat_emb = inp["cat_table"].astype(f32)[category]
    conf_p = conf @ inp["conf_w"].astype(f32) + inp["conf_b"].astype(f32)
    center_p = np.stack([cx, cy], axis=-1) @ inp["center_w"].astype(f32) + inp["center_b"].astype(f32)
    cam_emb = inp["cam_table"].astype(f32).reshape(1, 1, NCAM, 1, D)
    tok = (geom_p + cat_emb + conf_p + center_p + cam_emb) * float(inp["scale"])
    tok = np.where(presence == 0, inp["missing_emb"].astype(f32)[0], tok)
    out = np.concatenate([dist_tok.reshape(B, T * NCAM, D),
                          tok.reshape(B, T * NCAM * NB, D)], axis=1)
    return out.astype(np.float32)


def _run(inputs, trace=False, tmpdir=None):
    from concourse.bass_utils import run_bass_kernel_spmd

    dbz = bool(np.all(np.asarray(inputs["dist_b"]) == 0))
    key = ("nc", dbz)
    if key not in _CACHE:
        _CACHE[key] = _build_nc(dist_b_zero=dbz)
    nc = _CACHE[key]

    fpks, bpk = _prep_inputs(inputs)
    in_maps = [{"fpk": fpks[c], "bpk": bpk} for c in range(NCORES)]
    res = run_bass_kernel_spmd(nc, in_maps, list(range(NCORES)),
                               trace=trace, tmpdir=tmpdir)
    out = np.concatenate([np.asarray(res.results[c]["out"])
                          for c in range(NCORES)], axis=0)
    return out.astype(np.float32), res


def kernel(**inputs):
    if not _fast_path_ok(inputs):
        return _numpy_fallback(inputs)
    out, _ = _run(inputs)
    return out


if __name__ == "__main__":
    import reference as ref
    inputs = {k: np.asarray(v) for k, v in ref.setup_inputs().items()}
    got = kernel(**inputs)
    exp = np.load("/tmp/expected.npy")
    d = got - exp
    print("rel fro:", np.linalg.norm(d) / np.linalg.norm(exp))
    print("absmax rel:", np.abs(d).max() / np.abs(exp).max())
